# revision 58
# baseline (speedup 1.0000x reference)
"""Trainium2 Bass kernel for nn_MultiHeadAttention (dense transformer block).

Reference computation (B=2 batches, N=2048 tokens, C=1024, H=16 heads, D=64):
    qkv  = x @ W_qkv.T + b_qkv
    q,k,v split into heads; attn = softmax(q @ k.T / sqrt(D)); o = attn @ v
    out  = o @ W_proj.T + b_proj

Sharding over 8 NeuronCores: batch x head-groups.  Core c handles batch
b = c//4 and the 4 heads [4*(c%4), 4*(c%4)+4).  Attention is computed fully
per (batch, head) on one core.  The output projection needs all heads, so
cores AllGather their head-pair outputs O^T within their 4-core batch group,
then each core computes the full projection for a distinct 512-token slice
of its batch.  Host concatenates the 8 slices.

All matmul operands are fp16 (full-rate on the PE regardless of moving-dim
size); accumulation stays fp32 in PSUM.  Structure per core:

  phase 1: q^T, k^T via weight-stationary matmuls ([od,n] layout, bias via
           per-partition DVE add); v in NATURAL [token, od] layout directly
           (x^T-chunk stationary), no PE transposes.  v bias is exact to
           fold through softmax (sum of weights = 1), so it is added to o^T
           at the end instead.
  phase 2: per (head, 512-query-chunk) unit: S^T tiles [j=128, i=512] ->
           exp on ACT -> transposed AV: out o[i=128, 65] accumulating over
           16 j-tiles with e-chunk stationary and [v | ones] moving (65-row
           matmuls, half the rows of the o^T formulation).  col 64 is the
           softmax denominator; normalize with per-partition reciprocal on
           DVE.  S/exp runs LOOKAHEAD units ahead of AV; leftover phase-1
           matmuls (v, q/k s1) are woven into the PE stream as fillers.
  phase 3: head-pair o tiles are PE-transposed into o^T [128, n] chunks
           (+v-bias via per-partition add) and AllGather'd per chunk so the
           first chunk's collective overlaps the second head-pair's
           attention.
  phase 4: projection of the core's own 512-token slice from the gathered
           o^T chunks; W_proj^T is prefetched at kernel start.
"""

import sys

sys.path.insert(0, "/opt/trn_rl_repo")

import numpy as np
import concourse.bass as bass
import concourse.tile as tile
from concourse import mybir, bacc
from concourse.bass_utils import run_bass_kernel_spmd

f32 = mybir.dt.float32
fp16 = mybir.dt.float16
i32 = mybir.dt.int32

# problem constants (hardcoded per contract)
B = 2
N = 2048
C = 1024
H = 16
D = C // H  # 64
SCALE = D ** -0.5

NCORES = 8
GROUPS = [[0, 1, 2, 3], [4, 5, 6, 7]]
HPC = H // 4    # heads per core = 4
ODC = HPC * D   # per-core o-dim slice = 256
TOKS = N // 4   # output token slice per core = 512

N_CT = C // 128   # 8 contraction chunks
N_JT = N // 128   # 16 key tiles
N_IC = N // 512   # 4 query chunks per head
NU = HPC * N_IC   # 16 (head, ic) units
LOOKAHEAD = 3
NEBUF = LOOKAHEAD + 1

# exp: ACT does 13/16 j-tiles per unit with exp(s-3); the rest go to DVE+Pool
# via the Schraudolph bit-trick exp (i32(A*s + B) bitcast as f32), keeping the
# ACT engine off the critical path.
EXP_OFFLOAD = {2, 5, 8, 11, 14}
EXP_OFF_DVE = {2  # this tile's post-copies run on DVE (Pool is near-full)
EXP_SHIFT = 3.0
A_SCH = 12102203.161561485                  # 2^23 / ln 2
B_SCH3 = 1064866805.0 - EXP_SHIFT * A_SCH   # minimax B, shifted by exp(-3)


def build_kernel(ag=True):
    nc = bacc.Bacc("TRN2", target_bir_lowering=False, debug=False,
                   num_devices=NCORES)

    # ---- DRAM I/O (fp16 activations/weights, f32 biases/output) ----
    xt = nc.dram_tensor("xt", [C, N], fp16, kind="ExternalInput").ap()
    wq_t = nc.dram_tensor("wq_t", [C, ODC], fp16, kind="ExternalInput").ap()
    wk_t = nc.dram_tensor("wk_t", [C, ODC], fp16, kind="ExternalInput").ap()
    wv_t = nc.dram_tensor("wv_t", [C, ODC], fp16, kind="ExternalInput").ap()
    bqk = nc.dram_tensor("bqk", [128, 4], f32, kind="ExternalInput").ap()
    bvc = nc.dram_tensor("bvc", [128, 2], f32, kind="ExternalInput").ap()
    wp_t = nc.dram_tensor("wp_t", [4 * ODC, C], fp16, kind="ExternalInput").ap()
    bp = nc.dram_tensor("bp", [C], f32, kind="ExternalInput").ap()
    y = nc.dram_tensor("y", [TOKS, C], fp16, kind="ExternalOutput").ap()

    with tile.TileContext(nc, pool_alloc_mode="queue") as tc:
        with (
            tc.tile_pool(name="consts", bufs=1) as consts,
            tc.tile_pool(name="persist", bufs=1) as persist,
            tc.tile_pool(name="dram", bufs=1, space="DRAM") as dram,
        ):
            # ---- constants / small inputs ----
            bqk_sb = consts.tile([128, 4], f32)
            bvc_sb = consts.tile([128, 2], f32)
            bp_bc = consts.tile([128, C], f32)
            ident = consts.tile([128, 128], fp16)
            nc.gpsimd.memset(ident, 0.0)
            nc.gpsimd.affine_select(
                out=ident, in_=ident,
                compare_op=mybir.AluOpType.not_equal,
                fill=1.0, base=0, pattern=[[-1, 128]], channel_multiplier=1,
            )

            # ---- persistent SBUF tensors ----
            # single queue, ordered by need: wk -> x cols 0:1024 -> wq -> wv
            # -> x cols 1024:2048 -> wp -> bp.  First k matmuls start after
            # just wk + half of x.
            w_sb = {}
            for name, t in (("q", wq_t), ("k", wk_t), ("v", wv_t)):
                w_sb[name] = persist.tile([128, N_CT, ODC], fp16, name=f"w_{name}")
            xt_sb = persist.tile([128, N_CT, N], fp16)
            xt_v = xt.rearrange("(ct p) n -> p ct n", p=128)
            wp_sb = persist.tile([128, 8, C], fp16)

            def w_dma(name):
                t = {"q": wq_t, "k": wk_t, "v": wv_t}[name]
                nc.sync.dma_start(
                    out=w_sb[name], in_=t.rearrange("(ct p) m -> p ct m", p=128))

            def x_dma(half):
                for ct in range(N_CT):
                    nc.sync.dma_start(
                        out=xt_sb[:, ct, half * 1024:(half + 1) * 1024],
                        in_=xt_v[:, ct, half * 1024:(half + 1) * 1024],
                    )

            w_dma("k")
            x_dma(0)
            nc.sync.dma_start(out=bqk_sb, in_=bqk)
            nc.sync.dma_start(out=bvc_sb, in_=bvc)
            w_dma("q")
            w_dma("v")
            x_dma(1)
            nc.sync.dma_start(
                out=wp_sb, in_=wp_t.rearrange("(od p) c -> p od c", p=128))
            nc.sync.dma_start(
                out=bp_bc,
                in_=bass.AP(tensor=bp.tensor, offset=bp.offset,
                            ap=[[0, 128]] + bp.ap),
            )

            qt_sb = persist.tile([128, 2, N], fp16)   # q^T; head h=(s, p//64)
            kt_sb = persist.tile([128, 2, N], fp16)   # k^T
            vp_sb = persist.tile([128, N_JT, HPC, 65], fp16)  # v natural + ones
            e_sb = persist.tile([128, NEBUF, N_JT, 512], fp16)  # exp(S^T)
            o_nat = persist.tile([128, N_JT, 2, 128], fp16)  # o natural, hp pairs
            ot_sb = persist.tile([128, 2, N], fp16)   # o^T head-pair chunks
            ogt = persist.tile([128, 2, 4, TOKS], fp16)  # gathered o^T slices

            ones_bf = consts.tile([128, 1], fp16)
            nc.vector.memset(ones_bf, 1.0)
            shift_sb = consts.tile([128, 1], f32)
            nc.vector.memset(shift_sb, -EXP_SHIFT)
            nc.vector.tensor_copy(
                out=vp_sb[:, :, :, 64:65],
                in_=ones_bf[:, 0:1].to_broadcast((128, N_JT, HPC, 1)),
            )

            y_acc = persist.tile([128, 8, 512], f32)  # proj pass-A partials

            with (
                tc.tile_pool(name="mm512", bufs=5, space="PSUM") as mm512,
                tc.tile_pool(name="o_psp", bufs=2, space="PSUM") as o_psp,
                tc.tile_pool(name="trp", bufs=1, space="PSUM") as trp,
                tc.tile_pool(name="rcp", bufs=4) as rcp,
                tc.tile_pool(name="schp", bufs=3) as schp,
            ):
                pmm = mm512
                ps_sp = mm512
                # ---- phase-1 emission helpers ----
                def emit_qk_block(name, s, half, n2):
                    """One [128,512] output block of q^T or k^T (8 ct matmuls
                    + per-partition bias add)."""
                    dst = qt_sb if name == "q" else kt_sb
                    col = (0 if name == "q" else 2) + s
                    nn = half * 1024 + n2 * 512
                    ps = pmm.tile([128, 512], f32, tag="mm", name="ps_qk")
                    for ct in range(N_CT):
                        nc.tensor.matmul(
                            ps,
                            lhsT=w_sb[name][:, ct, s * 128:(s + 1) * 128],
                            rhs=xt_sb[:, ct, nn:nn + 512],
                            start=(ct == 0), stop=(ct == N_CT - 1),
                        )
                    nc.vector.tensor_scalar_add(
                        out=dst[:, s, nn:nn + 512], in0=ps,
                        scalar1=bqk_sb[:, col:col + 1],
                    )

                def emit_v_head(h, tt_range):
                    """v natural for head h over token tiles tt_range:
                    out v[tok=128, d=64] per tile (x^T chunk stationary)."""
                    for tt in tt_range:
                        ps = pmm.tile([128, 512], f32, tag="mm",
                                      name="ps_v")[:, 0:64]
                        for ct in range(N_CT):
                            nc.tensor.matmul(
                                ps,
                                lhsT=xt_sb[:, ct, tt * 128:(tt + 1) * 128],
                                rhs=w_sb["v"][:, ct, h * 64:(h + 1) * 64],
                                start=(ct == 0), stop=(ct == N_CT - 1),
                            )
                        nc.vector.tensor_copy(out=vp_sb[:, tt, h, 0:64], in_=ps)

                def emit_S(u, av_interleave=None):
                    h, ic = divmod(u, N_IC)
                    s, hh = divmod(h, 2)
                    for jt in range(N_JT):
                        ps = ps_sp.tile([128, 512], f32, tag="mm", name="ps_s")
                        nc.tensor.matmul(
                            ps,
                            lhsT=kt_sb[hh * 64:(hh + 1) * 64, s,
                                       jt * 128:(jt + 1) * 128],
                            rhs=qt_sb[hh * 64:(hh + 1) * 64, s,
                                      ic * 512:(ic + 1) * 512],
                            start=True, stop=True,
                        )
                        edst = e_sb[:, u % NEBUF, jt, :]
                        if jt in EXP_OFFLOAD:
                            # Schraudolph exp on DVE+Pool: i = A*s + B',
                            # bitcast to f32; keeps ACT below the PE rate.
                            schf = schp.tile([128, 512], f32, tag="schf",
                                             name="schf")
                            nc.vector.tensor_scalar(
                                out=schf, in0=ps, scalar1=A_SCH,
                                scalar2=B_SCH3,
                                op0=mybir.AluOpType.mult,
                                op1=mybir.AluOpType.add)
                            schi = schp.tile([128, 512], i32, tag="schi",
                                             name="schi")
                            eng = (nc.vector if jt in EXP_OFF_DVE
                                   else nc.gpsimd)
                            eng.tensor_copy(out=schi, in_=schf)
                            eng.tensor_copy(out=edst,
                                            in_=schi[:].bitcast(f32))
                        else:
                            # exp(s - 3): constant shift keeps e in fp16
                            # range; softmax is shift-invariant.
                            nc.scalar.activation(
                                out=edst, in_=ps,
                                func=mybir.ActivationFunctionType.Exp,
                                bias=shift_sb[:, 0:1],
                            )
                        if av_interleave is not None and jt % 4 == 1:
                            emit_AV(av_interleave, it2_range=[jt // 4])

                def emit_AV(u, it2_range=range(4)):
                    h, ic = divmod(u, N_IC)
                    for it2 in it2_range:
                        o_ps = o_psp.tile([128, 65], f32, tag="o_ps", name="o_ps")
                        for jt in range(N_JT):
                            nc.tensor.matmul(
                                o_ps,
                                lhsT=e_sb[:, u % NEBUF, jt,
                                          it2 * 128:(it2 + 1) * 128],
                                rhs=vp_sb[:, jt, h, :],
                                start=(jt == 0), stop=(jt == N_JT - 1),
                            )
                        r = rcp.tile([128, 1], f32, tag="r", name="r")
                        nc.vector.reciprocal(out=r, in_=o_ps[:, 64:65])
                        nc.vector.tensor_scalar_mul(
                            out=o_nat[:, ic * 4 + it2, h // 2,
                                      (h % 2) * 64:(h % 2) * 64 + 64],
                            in0=o_ps[:, 0:64], scalar1=r,
                        )
                        if h % 2 == 1:
                            # head pair complete for this i-tile: transpose to
                            # o^T, add v bias, and stream the AG input chunk
                            hp = h // 2
                            itile = ic * 4 + it2
                            ptr = trp.tile([128, 128], fp16, tag="tr",
                                           name="ptr")
                            nc.tensor.transpose(
                                ptr, in_=o_nat[:, itile, hp, :], identity=ident)
                            nc.vector.tensor_scalar_add(
                                out=ot_sb[:, hp,
                                          itile * 128:(itile + 1) * 128],
                                in0=ptr, scalar1=bvc_sb[:, hp:hp + 1],
                            )
                            if itile % 4 == 3:
                                nc.sync.dma_start(
                                    out=ag_ins[hp][:, (itile - 3) * 128:
                                                   (itile + 1) * 128],
                                    in_=ot_sb[:, hp, (itile - 3) * 128:
                                              (itile + 1) * 128],
                                )

                def emit_projA(tt, nc2):
                    """First half of the projection (hp0 od-chunks from the
                    first AllGather) accumulated into SBUF with the bias,
                    woven into late phase-2 PE slack."""
                    ps = pmm.tile([128, 512], f32, tag="mm", name="ps_pa")
                    for r_ in range(4):
                        nc.tensor.matmul(
                            ps,
                            lhsT=ogt[:, 0, r_, tt * 128:(tt + 1) * 128],
                            rhs=wp_sb[:, r_, nc2 * 512:(nc2 + 1) * 512],
                            start=(r_ == 0), stop=(r_ == 3),
                        )
                    nc.vector.tensor_add(
                        out=y_acc[:, tt * 2 + nc2, :], in0=ps,
                        in1=bp_bc[:, nc2 * 512:(nc2 + 1) * 512],
                    )

                ag_ins = [dram.tile([128, N], fp16, name="ag_in0"),
                          dram.tile([128, N], fp16, name="ag_in1")]

                def emit_hp_tail(hp):
                    """Launch the AllGather for a completed o^T chunk (the
                    chunk was streamed to DRAM by emit_AV); pull back this
                    core's 512-token slice of each peer's rows."""
                    ag_in = ag_ins[hp]
                    pid = nc.partition_id()
                    tok0 = (pid % 4) * TOKS
                    if ag:
                        ag_out = dram.tile([512, N], fp16, name=f"ag_out{hp}")
                        nc.gpsimd.collective_compute(
                            "AllGather",
                            mybir.AluOpType.bypass,
                            ins=[ag_in[:].opt()],
                            outs=[ag_out[:].opt()],
                            replica_groups=GROUPS,
                        )
                        for r_ in range(4):
                            nc.sync.dma_start(
                                out=ogt[:, hp, r_, :],
                                in_=ag_out.rearrange("(r p) n -> p r n",
                                                     p=128)[
                                    :, r_, bass.ds(tok0, TOKS)],
                            )
                    else:
                        # model path: same bytes moved, no collective
                        for r_ in range(4):
                            nc.sync.dma_start(
                                out=ogt[:, hp, r_, :],
                                in_=ag_in[:, bass.ds(tok0, TOKS)],
                            )

                # ---- interleaved emission schedule ----
                # startup: minimal set before S(u0) = all of k_s0 + the first
                # q_s0 block; v(h0) front half fills the x-half1 DMA wait.
                emit_qk_block("k", 0, 0, 0)
                emit_qk_block("k", 0, 0, 1)
                emit_qk_block("q", 0, 0, 0)
                emit_v_head(0, range(0, 8))
                emit_qk_block("k", 0, 1, 0)
                emit_qk_block("k", 0, 1, 1)

                # filler queue, index = slot, ordered by deadline:
                # q_s0 blocks just-in-time for units 1-3; k/q s1 before
                # unit 8..11; v(h) before its first AV group.
                qb = emit_qk_block
                fillers = [
                    lambda: (qb("q", 0, 0, 1), emit_v_head(0, range(8, 16))),
                    lambda: (qb("q", 0, 1, 0), emit_v_head(1, range(0, 8))),
                    lambda: (qb("q", 0, 1, 1), emit_v_head(1, range(8, 16))),
                    lambda: qb("k", 1, 0, 0),
                    lambda: qb("k", 1, 0, 1),
                    lambda: qb("k", 1, 1, 0),
                    lambda: (qb("k", 1, 1, 1), qb("q", 1, 0, 0)),
                    lambda: (qb("q", 1, 0, 1), emit_v_head(2, range(0, 8))),
                    lambda: (qb("q", 1, 1, 0), emit_v_head(2, range(8, 16))),
                    lambda: (qb("q", 1, 1, 1), emit_v_head(3, range(0, 8))),
                    lambda: emit_v_head(3, range(8, 16)),
                ]
                for slot in range(NU + LOOKAHEAD):
                    u = slot - LOOKAHEAD
                    if slot < NU:
                        emit_S(slot, av_interleave=u if u >= 0 else None)
                    elif u >= 0:
                        emit_AV(u)
                    if slot < len(fillers):
                        fillers[slot]()
                    if u >= 0:
                        if u == 7:
                            emit_hp_tail(0)
                        elif u == 15:
                            emit_hp_tail(1)
                            emit_projA(3, 0)
                            emit_projA(3, 1)
                    if 12 <= slot <= 14:
                        k2 = (slot - 12) * 2
                        emit_projA(k2 // 2, k2 % 2)
                        k2 += 1
                        emit_projA(k2 // 2, k2 % 2)

            # ---------- phase 4: projection pass-B (hp1 od-chunks) ----------
            with (
                tc.tile_pool(name="p4ps", bufs=3, space="PSUM") as p4ps,
                tc.tile_pool(name="p4y", bufs=3) as p4y,
            ):
                for tt in range(TOKS // 128):
                    for nc2 in range(C // 512):
                        ps_y = p4ps.tile([128, 512], f32, tag="p4ps", name="ps_y")
                        for r_ in range(4):
                            nc.tensor.matmul(
                                ps_y,
                                lhsT=ogt[:, 1, r_, tt * 128:(tt + 1) * 128],
                                rhs=wp_sb[:, 4 + r_, nc2 * 512:(nc2 + 1) * 512],
                                start=(r_ == 0), stop=(r_ == 3),
                            )
                        y_sb = p4y.tile([128, 512], fp16, tag="y_sb", name="y_sb")
                        nc.vector.tensor_add(
                            out=y_sb, in0=ps_y,
                            in1=y_acc[:, tt * 2 + nc2, :],
                        )
                        nc.sync.dma_start(
                            out=y[tt * 128:(tt + 1) * 128,
                                  nc2 * 512:(nc2 + 1) * 512],
                            in_=y_sb,
                        )

    nc.compile()
    return nc


_CACHE = {}


def _get_nc():
    if "nc" not in _CACHE:
        _CACHE["nc"] = build_kernel()
    return _CACHE["nc"]


def _bf(a):
    return np.ascontiguousarray(a.astype(np.float16))


def make_in_maps(x, W_qkv, b_qkv, W_proj, b_proj):
    x = np.asarray(x, dtype=np.float32)
    W_qkv = np.asarray(W_qkv, dtype=np.float32)
    b_qkv = np.asarray(b_qkv, dtype=np.float32)
    W_proj = np.asarray(W_proj, dtype=np.float32)
    b_proj = np.asarray(b_proj, dtype=np.float32)

    Wq = W_qkv[0:C] * SCALE
    Wk = W_qkv[C:2 * C]
    Wv = W_qkv[2 * C:3 * C]
    bq = b_qkv[0:C] * SCALE
    bk = b_qkv[C:2 * C]
    bv = b_qkv[2 * C:3 * C]

    # W_proj^T rows permuted to AllGather output order: chunk j = hp*4 + r
    # holds global od rows r*256 + hp*128 .. +128.
    wp_rows = np.empty((4 * ODC, C), dtype=np.float32)
    for hp in range(2):
        for r in range(4):
            j = hp * 4 + r
            src = r * ODC + hp * 128
            wp_rows[j * 128:(j + 1) * 128] = W_proj.T[src:src + 128]
    wp_bf = _bf(wp_rows)

    xt_by_b = [_bf(x[b].T) for b in range(B)]  # [C, N] fp16
    per_g = []
    for g in range(4):
        rows = slice(g * ODC, (g + 1) * ODC)
        bqk_cols = np.stack(
            [bq[rows][0:128], bq[rows][128:256],
             bk[rows][0:128], bk[rows][128:256]], axis=1
        ).astype(np.float32)  # [128, 4]
        bvc_cols = np.stack(
            [bv[rows][0:128], bv[rows][128:256]], axis=1
        ).astype(np.float32)  # [128, 2]
        per_g.append({
            "wq_t": _bf(Wq[rows].T),
            "wk_t": _bf(Wk[rows].T),
            "wv_t": _bf(Wv[rows].T),
            "bqk": np.ascontiguousarray(bqk_cols),
            "bvc": np.ascontiguousarray(bvc_cols),
        })
    in_maps = []
    for core in range(NCORES):
        b = core // 4
        g = core % 4
        in_maps.append({
            "xt": xt_by_b[b],
            **per_g[g],
            "wp_t": wp_bf,
            "bp": b_proj,
        })
    return in_maps


def kernel(x, W_qkv, b_qkv, W_proj, b_proj):
    nc = _get_nc()
    in_maps = make_in_maps(x, W_qkv, b_qkv, W_proj, b_proj)
    res = run_bass_kernel_spmd(nc, in_maps, list(range(NCORES)))

    out = np.empty((B, N, C), dtype=np.float32)
    for core in range(NCORES):
        b = core // 4
        g = core % 4
        out[b, g * TOKS:(g + 1) * TOKS, :] = res.results[core]["y"].astype(np.float32)
    return out
